# revision 9
# baseline (speedup 1.0000x reference)
"""Dilated attention (LongNet-style) Trainium2 Bass kernel.

Problem: q/k/v [b=2, seq=8192, h=12, d=64], 3 dilation groups of 4 heads:
  group 0: segment 2048, rate 1, off 0, heads 0-3   -> 4 segments/batch
  group 1: segment 4096, rate 2, off 1, heads 4-7   -> 2 segments/batch
  group 2: segment 8192, rate 4, off 2, heads 8-11  -> 1 segment/batch
Every (batch, head, segment) is an independent causal attention of shape
[m=2048, k=2048, d=64]; there are 56 such problems (32+16+8), all equal cost.

Sharding: 8 cores = 2 batches x 4 "head triples". Core c owns batch c//4 and
heads {j, 4+j, 8+j} (j = c%4) -> 4+2+1 = 7 problems per core, and every head
lives entirely on one core, so the final seq-sum renormalization is local
(no collectives).

On-core layout ("transposed"): S^T[k, m] = K Q^T computed per (k-chunk=128,
m-tile=512); exp via ACT; PV accumulates O^T[d, m] with lhsT = V_aug
([128, 65], last column ones => row 64 of O^T is the softmax denominator l[m]).
Causality: k-chunks fully above the diagonal are skipped, band chunks are
column-trimmed and their leading 128x128 triangle gets an additive -1e9 mask
on the PSUM scores pre-exp.

Precision: QK^T scores in fp32 (4 cyc/row, packed x2 across PE row groups =>
2 cyc/col effective). The PV matmul reads P and V_aug as float32r (FP22
truncation on PE, 1 cyc/row): numpy emulation of FP22 shows final rel err
7.1e-3 vs the 2e-2 gate (fp32 everywhere: 1.1e-4; bf16 PV: 0.94 -- the
seq-sum renorm divides by a heavily-cancelled sum, which amplifies any
per-position-independent noise ~90x, so P/V/l must stay at fp22+ precision
while rl and the staging math stay exact fp32).

Normalization chain per m-tile: DVE reciprocal of the l row (PSUM), gpsimd
partition_broadcast of 1/l across the 64 d-partitions (frees the PE of the
baseline's broadcast matmul and PSUM bank), then one fused DVE
tensor_tensor_reduce: x^T = O^T * bc AND its free-axis partial seq-sum, in a
single instruction. Per-head seq-sum renorm stays local and is emitted as
soon as a head's problems complete.

Scores for a 2-chunk pair land in one [128, 1024] PSUM super-tile (2 banks)
so one ACT exp instruction covers 2 chunks (ACT per-instruction overhead
~130ns is the 2nd-order cost after the matmuls).

PE packing: QK^T contracts over d=64 (half the PE array), so q/k are
duplicated onto both partition halves and adjacent chunks issue on row
groups (0,0)/(64,0) -> concurrent execution (measured exact on HW).
"""

import numpy as np

B, SEQ, H, D = 2, 8192, 12, 64
NP = 7            # problems per core
M = 2048          # dilated positions per problem
MT = 512          # m-tile width
KC = 128          # k-chunk (partition) width
NMT = M // MT     # 4 m-tiles
NKC = M // KC     # 16 k-chunks
SCALE = 0.125     # 1/sqrt(64)

QK_F32R = False   # fp32r QK^T scores (emulated rel err 1.07e-2 vs 7.1e-3)
SUPER = False     # 2-chunk [128,1024] PSUM super-tiles + batched exp
BCAST_GPSIMD = False  # 1/l partition broadcast on gpsimd vs PE matmul
TTR = False       # fused DVE tensor_tensor_reduce for x-mul + seq-sum

_CACHE = {}


def _core_problems(core):
    """The 7 (head, positions) problems for a core; batch = core//4."""
    j = core % 4
    probs = []
    for p in range(4):
        probs.append((j, p * 2048 + np.arange(2048)))
    for p in range(2):
        probs.append((4 + j, p * 4096 + 1 + 2 * np.arange(2048)))
    probs.append((8 + j, 2 + 4 * np.arange(2048)))
    return probs


# head -> list of problem indices on its core
HEAD_GROUPS = ((0, 1, 2, 3), (4, 5), (6,))


def _import_concourse():
    try:
        import concourse  # noqa: F401
    except ImportError:
        import sys

        for p in ("/opt/trn_rl_repo", "/root/.axon_site/_ro/trn_rl_repo"):
            if p not in sys.path:
                sys.path.append(p)


def _build_program(causal, reps=1, debug_stage=False):
    """Build the SPMD program. reps>1 wraps the compute in a hardware loop
    (timing-only variant; the deliverable path uses reps=1)."""
    _import_concourse()
    import contextlib

    import concourse.bass as bass  # noqa: F401
    import concourse.tile as tile
    from concourse import bacc, mybir

    F32 = mybir.dt.float32
    F32R = mybir.dt.float32r
    QKDT = F32R if QK_F32R else F32

    nc = bacc.Bacc()

    # q and k share one tensor: [p, :, 0:2048]=Q^T, [p, :, 2048:4096]=K^T.
    # DMA'd twice (partitions 0:64 and 64:128) so even k-chunks run on PE row
    # group 0 and odd chunks on row group 64.
    # float32r tensors hold fp32 bits; the BIR verifier requires the
    # producer's output dtype (DMA for va/qkt, ACT exp for pt) to be f32r
    # when a f32r matmul consumes it.
    qkT_d = nc.dram_tensor("qkT", [NP, D, 2 * M], QKDT, kind="ExternalInput")
    F16 = mybir.dt.float16
    vAh_d = nc.dram_tensor("vAh", [KC, NP, NKC, D + 1], F16, kind="ExternalInput")
    vAl_d = nc.dram_tensor("vAl", [KC, NP, NKC, D + 1], F16, kind="ExternalInput")
    out_d = nc.dram_tensor("out", [NP, D, M], F32, kind="ExternalOutput")
    dbg_d = (
        nc.dram_tensor("dbg", [D, NP * NMT], F32, kind="ExternalOutput")
        if debug_stage else None
    )

    # additive causal mask for the leading 128x128 triangle of band chunks:
    # 0 where col>=row (valid), -1e9 otherwise (exp underflows to exactly 0).
    mneg = np.where(
        np.arange(KC)[None, :] >= np.arange(KC)[:, None], 0.0, -1e9
    ).astype(np.float32)
    mask_d = nc.inline_tensor(mneg, name="cmask")

    with tile.TileContext(nc) as tc:
        import contextlib as _ctx
        CW = 2 * MT if SUPER else MT   # score-tile width in chunks' columns
        s_bufs = (3 if BCAST_GPSIMD else 2) if SUPER else (5 if BCAST_GPSIMD else 5)
        with (
            tc.tile_pool(name="qk", bufs=4) as qk_pool,
            tc.tile_pool(name="pt", bufs=10 if SUPER else 17) as p_pool,
            tc.tile_pool(name="small", bufs=1) as small_pool,
            tc.tile_pool(name="stage", bufs=1) as stage_pool,
            tc.tile_pool(name="rl", bufs=3) as rl_pool,
            tc.tile_pool(name="bc", bufs=3) as bc_pool,
            tc.tile_pool(name="spsum", bufs=s_bufs, space="PSUM") as s_psum,
            tc.tile_pool(name="opsum", bufs=2, space="PSUM") as o_psum,
            (_ctx.nullcontext() if BCAST_GPSIMD else
             tc.tile_pool(name="bpsum", bufs=1, space="PSUM")) as b_psum,
        ):
            mask_sb = small_pool.tile([KC, KC], F32)
            nc.sync.dma_start(out=mask_sb, in_=mask_d[:])
            if not BCAST_GPSIMD:
                ones_sb = small_pool.tile([1, D], F32)
                nc.vector.memset(ones_sb, 1.0)

            # all value tensors upfront; split per problem AND per
            # partition-slab so transfers spread across HWDGE queues
            # (per-queue bandwidth is ~10-15 GB/s)
            vah = small_pool.tile([KC, NP, NKC, D + 1], F16)
            val = small_pool.tile([KC, NP, NKC, D + 1], F16)
            for p in range(NP):
                for s in range(4):
                    sl = slice(s * 32, (s + 1) * 32)
                    nc.sync.dma_start(
                        out=vah[sl, p, :, :], in_=vAh_d[sl, p, :, :]
                    )
                    nc.sync.dma_start(
                        out=val[sl, p, :, :], in_=vAl_d[sl, p, :, :]
                    )

            # O_norm^T staging: [d=64, problem, m=2048]
            stage = stage_pool.tile([D, NP, M], F32)
            # per (problem, m-tile) partial seq-sums
            dsums = small_pool.tile([D, NP * NMT], F32)

            # head renorm emitted as soon as its problems complete, so the
            # tail DVE/DMA work overlaps later problems' PE work
            done_after = {3: HEAD_GROUPS[0], 5: HEAD_GROUPS[1], 6: HEAD_GROUPS[2]}

            rep_loop = (
                tc.For_i(0, reps, 1) if reps > 1 else contextlib.nullcontext()
            )
            with rep_loop:
              for p in range(NP):
                # 8 slab DMAs (4 per duplicated half) to spread descriptor
                # latency across HWDGE queues
                qkt = qk_pool.tile([2 * D, 2 * M], QKDT, tag="qkt")
                SL = D // 4
                for h in range(2):
                    for s in range(4):
                        nc.sync.dma_start(
                            out=qkt[h * D + s * SL:h * D + (s + 1) * SL, :],
                            in_=qkT_d[p, s * SL:(s + 1) * SL, :],
                        )

                for j in range(NMT):
                    m0 = j * MT
                    if causal:
                        # band chunks (4j..4j+3, trimmed) first so their
                        # serial S->mask->exp chains hide behind the full
                        # chunks' matmuls; band 4j is full-width, so the
                        # PV accumulation start still covers the whole bank
                        chunks = [(4 * j + i, KC * i) for i in range(4)]
                        chunks += [(kc, 0) for kc in range(4 * j)]
                    else:
                        chunks = [(kc, 0) for kc in range(NKC)]

                    # phase 1: S matmuls as row-group pairs; with SUPER,
                    # a 2-chunk pair lands in one [128, 1024] PSUM tile and
                    # (when untrimmed) one ACT exp covers both chunks
                    pts = []
                    gsz = 2 if SUPER else 1
                    for s0 in range(0, len(chunks), gsz):
                        pair = chunks[s0:s0 + gsz]
                        st = s_psum.tile([KC, CW], F32, tag="st")
                        pt = p_pool.tile([KC, CW], F16, tag="pt")
                        for t, (kc, coff) in enumerate(pair):
                            r0 = ((s0 + t) % 2) * D  # alternate row groups
                            nc.tensor.matmul(
                                st[:, t * MT + coff:(t + 1) * MT],
                                qkt[r0:r0 + D, M + kc * KC:M + (kc + 1) * KC],
                                qkt[r0:r0 + D, m0 + coff:m0 + MT],
                                start=True,
                                stop=True,
                            )
                        for t, (kc, coff) in enumerate(pair):
                            if causal and kc >= 4 * j:
                                # mask this band chunk's leading triangle
                                c0 = t * MT + coff
                                nc.vector.tensor_add(
                                    st[:, c0:c0 + KC], st[:, c0:c0 + KC], mask_sb
                                )
                        trimmed = any(coff for _, coff in pair)
                        if trimmed:
                            for t, (kc, coff) in enumerate(pair):
                                nc.scalar.activation(
                                    pt[:, t * MT + coff:(t + 1) * MT],
                                    st[:, t * MT + coff:(t + 1) * MT],
                                    mybir.ActivationFunctionType.Exp,
                                    scale=SCALE,
                                )
                        else:
                            w = len(pair) * MT
                            nc.scalar.activation(
                                pt[:, :w],
                                st[:, :w],
                                mybir.ActivationFunctionType.Exp,
                                scale=SCALE,
                            )
                        for t, (kc, coff) in enumerate(pair):
                            pts.append((pt, t, kc, coff))

                    # phase 2: PV accumulation over all chunks (fp32r: FP22
                    # products, fp32 PSUM accumulate, 1 cyc/row)
                    ot = o_psum.tile([D + 1, MT], F32)
                    for idx, (pt, t, kc, coff) in enumerate(pts):
                        for vi, vv in enumerate((vah, val)):
                            nc.tensor.matmul(
                                ot[:, coff:],
                                vv[:, p, kc, :],
                                pt[:, t * MT + coff:(t + 1) * MT],
                                start=(idx == 0 and vi == 0),
                                stop=(idx == len(pts) - 1 and vi == 1),
                            )

                    # softmax normalization: x^T = O^T / l[m] (row 64 of ot).
                    # 1/l broadcast across the 64 d-partitions on gpsimd (the
                    # PE and DVE stay free); one fused DVE op multiplies and
                    # accumulates the per-m-tile seq-sum.
                    rl = rl_pool.tile([1, MT], F32, tag="rl")
                    nc.vector.reciprocal(rl, ot[D:D + 1, :])
                    bc = bc_pool.tile([D, MT], F32, tag="bc")
                    if BCAST_GPSIMD:
                        nc.gpsimd.partition_broadcast(bc, rl, channels=D)
                    else:
                        bcp = b_psum.tile([D, MT], F32)
                        nc.tensor.matmul(bcp, ones_sb, rl, start=True, stop=True)
                        nc.vector.tensor_copy(bc, bcp)
                    dst = stage[:, p, m0:m0 + MT]
                    if TTR:
                        nc.vector.tensor_tensor_reduce(
                            out=dst,
                            in0=ot[:D, :],
                            in1=bc,
                            scale=1.0,
                            scalar=0.0,
                            op0=mybir.AluOpType.mult,
                            op1=mybir.AluOpType.add,
                            accum_out=dsums[:, p * NMT + j:p * NMT + j + 1],
                        )
                    else:
                        nc.vector.tensor_mul(dst, ot[:D, :], bc)
                        nc.vector.reduce_sum(
                            out=dsums[:, p * NMT + j:p * NMT + j + 1],
                            in_=dst,
                            axis=mybir.AxisListType.X,
                        )

                if debug_stage:
                    for s in range(4):
                        sl = slice(s * 16, (s + 1) * 16)
                        nc.gpsimd.dma_start(
                            out=out_d[p, sl, :], in_=stage[sl, p, :]
                        )
                    continue
                # per-head seq-sum renorm + output, as soon as the head is done
                if p in done_after:
                    probs = done_after[p]
                    denom = rl_pool.tile([D, 1], F32, tag="dn")
                    lo, hi = probs[0] * NMT, (probs[-1] + 1) * NMT
                    nc.vector.reduce_sum(
                        out=denom, in_=dsums[:, lo:hi], axis=mybir.AxisListType.X
                    )
                    rden = rl_pool.tile([D, 1], F32, tag="rd")
                    nc.vector.reciprocal(rden, denom)
                    for pp in probs:
                        nc.vector.tensor_scalar_mul(
                            stage[:, pp, :], stage[:, pp, :], rden
                        )
                        # 4 slab DMAs -> 4 parallel queues, issued from the
                        # gpsimd queue so output transfers never serialize
                        # behind the sync queue's input prefetch slot-waits
                        # (SP is strict FIFO)
                        for s in range(4):
                            sl = slice(s * 16, (s + 1) * 16)
                            nc.gpsimd.dma_start(
                                out=out_d[pp, sl, :], in_=stage[sl, pp, :]
                            )
              if debug_stage:
                nc.sync.dma_start(out=dbg_d[:], in_=dsums)

    nc.finalize()
    return nc


def _shard_inputs(query, key, value):
    """Build the 8 per-core input maps from full inputs."""
    in_maps = []
    for core in range(8):
        b = core // 4
        qkT = np.empty((NP, D, 2 * M), np.float32)
        vA = np.empty((KC, NP, NKC, D + 1), np.float32)
        vA[..., D] = 1.0
        for p, (h, pos) in enumerate(_core_problems(core)):
            s, e, st = pos[0], pos[-1] + 1, (pos[1] - pos[0])
            qkT[p, :, :M] = query[b, s:e:st, h, :].T
            qkT[p, :, M:] = key[b, s:e:st, h, :].T
            # vA[i, p, c, :64] = V[c*128 + i]
            vA[:, p, :, :D] = value[b, s:e:st, h, :].reshape(NKC, KC, D).transpose(1, 0, 2)
        vAh = vA.astype(np.float16)
        vAl = (vA - vAh.astype(np.float32)).astype(np.float16)
        in_maps.append({"qkT": qkT, "vAh": vAh, "vAl": vAl})
    return in_maps


def _unshard(results):
    out = np.zeros((B, SEQ, H, D), np.float32)
    for core in range(8):
        b = core // 4
        o = results[core]["out"]  # [NP, 64, 2048]
        for p, (h, pos) in enumerate(_core_problems(core)):
            s, e, st = pos[0], pos[-1] + 1, (pos[1] - pos[0])
            out[b, s:e:st, h, :] = o[p].T
    return out


def kernel(query, key, value, causal):
    _import_concourse()
    from concourse.bass_utils import run_bass_kernel_spmd

    query = np.asarray(query, np.float32)
    key = np.asarray(key, np.float32)
    value = np.asarray(value, np.float32)
    causal = bool(int(np.asarray(causal)))

    if causal not in _CACHE:
        _CACHE[causal] = _build_program(causal)
    nc = _CACHE[causal]

    in_maps = _shard_inputs(query, key, value)
    res = run_bass_kernel_spmd(nc, in_maps, core_ids=list(range(8)))
    return _unshard(res.results)


# revision 24
# speedup vs baseline: 1.6000x; 1.6000x over previous
"""Dilated attention (LongNet-style) Trainium2 Bass kernel.

Problem: q/k/v [b=2, seq=8192, h=12, d=64], 3 dilation groups of 4 heads:
  group 0: segment 2048, rate 1, off 0, heads 0-3   -> 4 segments/batch
  group 1: segment 4096, rate 2, off 1, heads 4-7   -> 2 segments/batch
  group 2: segment 8192, rate 4, off 2, heads 8-11  -> 1 segment/batch
Every (batch, head, segment) is an independent causal attention of shape
[m=2048, k=2048, d=64]; there are 56 such problems (32+16+8), all equal cost.

Sharding: 8 cores = 2 batches x 4 "head triples". Core c owns batch c//4 and
heads {j, 4+j, 8+j} (j = c%4) -> 4+2+1 = 7 problems per core, and every head
lives entirely on one core, so the final seq-sum renormalization is local
(no collectives).

On-core layout ("transposed"): S^T[k, m] = K Q^T computed per (k-chunk=128,
m-tile=512); exp via ACT; PV accumulates O^T[d, m] with an augmented ones
column in V so row 64 of O^T is the softmax denominator l[m]. Causality:
k-chunks fully above the diagonal are skipped, band chunks are column-trimmed
and their leading 128x128 triangle gets an additive -1e9 mask pre-exp.

Precision design (the final out/sum(out) renorm divides by a heavily
cancelled sum, amplifying any per-position-INDEPENDENT noise in the
attention outputs ~2000x into the result, while common-mode/truncation
error cancels in the ratio -- measured on HW, not just theory):
  - QK^T in float32r (FP22 operands, 1 cyc/col; fp32 is 4): score noise
    ~5e-4 independent -> contributes ~1e-2 of the final 1.46e-2 rel err.
  - exp -> P in fp16 (ACT converts; RNE, rel 2.4e-4; range safe since
    |s|*scale <= ~6).
  - PV with V split as V_hi + V_lo (two fp16 matmuls, 2 cyc/col total):
    fp16 x fp16 products are EXACT on the PE (11x11 bits < 24-bit
    accumulator), so x inherits only P's representation noise, which hits
    numerator and denominator coherently and mostly cancels. A single
    fp32r PV instead puts ~1.4e-4 of per-product rounding noise in x and
    fails at 2.5e-1 (measured); bf16 fails at 0.9.
  - l from the ones column is exact-product too; rl, the broadcast, the
    x-multiply, seq-sums and renorm all stay fp32.

Normalization per m-tile: DVE reciprocal of the l row, a K=1 fp32 PE matmul
broadcasts 1/l across the 64 d-partitions (DVE cannot broadcast across
partitions; gpsimd partition_broadcast costs ~250us/instr in loop mode),
then DVE multiply + free-axis reduce accumulate the per-head seq-sums.
The broadcast matmul of m-tile j is emitted after m-tile j+1's S matmuls
("deferred normalize") so the in-order PE never stalls on the DVE
reciprocal chain. Per-head renorm + output DMA are emitted as soon as the
last problem of a head completes.

DMA (the loop-amortized bottleneck): q/k are fetched ONCE per problem
([64, 4096] fp32, 1 MB) with slab DMAs split across both HWDGE engines
(SP + ACT); all S matmuls run on PE row group 0 (f32r at 1 cyc/col makes
the old duplicate-and-pack scheme a net loss: it doubled input bytes to
save 3.6us/problem of PE). V_hi/V_lo (fp16) load up front outside the
loop. Outputs are cast fp32->fp16 in flight by the gpsimd SWDGE (halving
output bytes; fp16 output rounding is ~12*2^-11 << the 2e-2 gate).

Measured (8 cores, loop-amortized over 800 reps): ~460 us/iter vs 736 us
baseline; cost model 255 us vs 476 us baseline; HW rel err 1.457e-2
(gate 2e-2). CoreSim validates structure; numerics were validated by a
numpy FP22/fp16 emulator plus HW microbenchmarks (see transcript).
"""

import numpy as np

B, SEQ, H, D = 2, 8192, 12, 64
NP = 7            # problems per core
M = 2048          # dilated positions per problem
MT = 512          # m-tile width
KC = 128          # k-chunk (partition) width
NMT = M // MT     # 4 m-tiles
NKC = M // KC     # 16 k-chunks
SCALE = 0.125     # 1/sqrt(64)

QK_F32R = True    # fp32r QK^T scores (HW rel err 1.46e-2 vs 7.2e-3 fp32)

_CACHE = {}


def _core_problems(core):
    """The 7 (head, positions) problems for a core; batch = core//4."""
    j = core % 4
    probs = []
    for p in range(4):
        probs.append((j, p * 2048 + np.arange(2048)))
    for p in range(2):
        probs.append((4 + j, p * 4096 + 1 + 2 * np.arange(2048)))
    probs.append((8 + j, 2 + 4 * np.arange(2048)))
    return probs


# head -> list of problem indices on its core
HEAD_GROUPS = ((0, 1, 2, 3), (4, 5), (6,))


def _import_concourse():
    try:
        import concourse  # noqa: F401
    except ImportError:
        import sys

        for p in ("/opt/trn_rl_repo", "/root/.axon_site/_ro/trn_rl_repo"):
            if p not in sys.path:
                sys.path.append(p)


def _build_program(causal, reps=1, debug_stage=False, timing_mode="full"):
    """Build the SPMD program. reps>1 wraps the compute in a hardware loop
    (timing-only variant; the deliverable path uses reps=1)."""
    _import_concourse()
    import contextlib

    import concourse.bass as bass  # noqa: F401
    import concourse.tile as tile
    from concourse import bacc, mybir

    F32 = mybir.dt.float32
    F16 = mybir.dt.float16
    F32R = mybir.dt.float32r
    QKDT = F32R if QK_F32R else F32

    nc = bacc.Bacc()

    # q and k share one tensor: [p, :, 0:2048]=Q^T, [p, :, 2048:4096]=K^T.
    # float32r holds plain fp32 bits; the BIR verifier requires the
    # producer's output dtype (the DMA) to be f32r when a f32r matmul
    # consumes it.
    qkT_d = nc.dram_tensor("qkT", [NP, D, 2 * M], QKDT, kind="ExternalInput")
    vAh_d = nc.dram_tensor("vAh", [KC, NP, NKC, D + 1], F16, kind="ExternalInput")
    vAl_d = nc.dram_tensor("vAl", [KC, NP, NKC, D + 1], F16, kind="ExternalInput")
    out_d = nc.dram_tensor("out", [NP, D, M], F16, kind="ExternalOutput")
    dbg_d = (
        nc.dram_tensor("dbg", [D, NP * NMT], F32, kind="ExternalOutput")
        if debug_stage else None
    )

    # additive causal mask for the leading 128x128 triangle of band chunks:
    # 0 where col>=row (valid), -1e9 otherwise (exp underflows to exactly 0).
    mneg = np.where(
        np.arange(KC)[None, :] >= np.arange(KC)[:, None], 0.0, -1e9
    ).astype(np.float32)
    mask_d = nc.inline_tensor(mneg, name="cmask")

    with tile.TileContext(nc) as tc:
        with (
            tc.tile_pool(name="qk", bufs=4) as qk_pool,
            tc.tile_pool(name="pt", bufs=17) as p_pool,
            tc.tile_pool(name="small", bufs=1) as small_pool,
            tc.tile_pool(name="stage", bufs=1) as stage_pool,
            tc.tile_pool(name="rl", bufs=3) as rl_pool,
            tc.tile_pool(name="bc", bufs=3) as bc_pool,
            tc.tile_pool(name="spsum", bufs=5, space="PSUM") as s_psum,
            tc.tile_pool(name="opsum", bufs=2, space="PSUM") as o_psum,
            tc.tile_pool(name="bpsum", bufs=1, space="PSUM") as b_psum,
        ):
            mask_sb = small_pool.tile([KC, KC], F32)
            nc.sync.dma_start(out=mask_sb, in_=mask_d[:])
            ones_sb = small_pool.tile([1, D], F32)
            nc.vector.memset(ones_sb, 1.0)

            # all value tensors upfront, split per problem and partition-slab
            # so transfers spread across queue slots
            vah = small_pool.tile([KC, NP, NKC, D + 1], F16)
            val = small_pool.tile([KC, NP, NKC, D + 1], F16)
            for p in range(NP):
                for s in range(4):
                    sl = slice(s * 32, (s + 1) * 32)
                    nc.sync.dma_start(
                        out=vah[sl, p, :, :], in_=vAh_d[sl, p, :, :]
                    )
                    nc.sync.dma_start(
                        out=val[sl, p, :, :], in_=vAl_d[sl, p, :, :]
                    )

            # O_norm^T staging: [d=64, problem, m=2048]
            stage = stage_pool.tile([D, NP, M], F32)
            # per (problem, m-tile) partial seq-sums
            dsums = small_pool.tile([D, NP * NMT], F32)

            # head renorm emitted as soon as its problems complete, so the
            # tail DVE/DMA work overlaps later problems' PE work
            done_after = {3: HEAD_GROUPS[0], 5: HEAD_GROUPS[1], 6: HEAD_GROUPS[2]}

            def load_qkt(p):
                # 4 slab DMAs, split across the two HWDGE engines (SP + ACT)
                # so the per-iteration input stream rides two DMA paths
                qkt = qk_pool.tile([D, 2 * M], QKDT, tag="qkt")
                SL = D // 4
                for s in range(4):
                    eng = nc.sync if s % 2 == 0 else nc.scalar
                    eng.dma_start(
                        out=qkt[s * SL:(s + 1) * SL, :],
                        in_=qkT_d[p, s * SL:(s + 1) * SL, :],
                    )
                return qkt

            pre_qkt = []
            if timing_mode == "noqkdma":
                for p in range(4):
                    pre_qkt.append(load_qkt(p))

            # deferred normalize: the PE-side broadcast matmul of m-tile j is
            # emitted after m-tile j+1's S matmuls, so the in-order PE never
            # stalls on the DVE reciprocal chain
            pending = []

            def normalize_flush():
                while pending:
                    p_, j_, ot_, rl_ = pending.pop(0)
                    bcp = b_psum.tile([D, MT], F32)
                    nc.tensor.matmul(bcp, ones_sb, rl_, start=True, stop=True)
                    bc = bc_pool.tile([D, MT], F32, tag="bc")
                    nc.vector.tensor_copy(bc, bcp)
                    dst = stage[:, p_, j_ * MT:(j_ + 1) * MT]
                    nc.vector.tensor_mul(dst, ot_[:D, :], bc)
                    nc.vector.reduce_sum(
                        out=dsums[:, p_ * NMT + j_:p_ * NMT + j_ + 1],
                        in_=dst,
                        axis=mybir.AxisListType.X,
                    )

            rep_loop = (
                tc.For_i(0, reps, 1) if reps > 1 else contextlib.nullcontext()
            )
            with rep_loop:
              for p in range(NP):
                if timing_mode == "noqkdma":
                    qkt = pre_qkt[p % 4]
                else:
                    qkt = load_qkt(p)
                if timing_mode == "dmaonly":
                    for s in range(4):
                        sl = slice(s * 16, (s + 1) * 16)
                        nc.gpsimd.dma_start(
                            out=out_d[p, sl, :], in_=qkt[sl, 0:M]
                        )
                    continue

                for j in range(NMT):
                    m0 = j * MT
                    if causal:
                        # band chunks (4j..4j+3, trimmed) first so their
                        # serial S->mask->exp chains hide behind the full
                        # chunks' matmuls; band 4j is full-width, so the
                        # PV accumulation start still covers the whole bank
                        chunks = [(4 * j + i, KC * i) for i in range(4)]
                        chunks += [(kc, 0) for kc in range(4 * j)]
                    else:
                        chunks = [(kc, 0) for kc in range(NKC)]

                    # phase 1: S matmuls (f32r, 1 cyc/col) + mask + exp->fp16
                    pts = []
                    for kc, coff in chunks:
                        st = s_psum.tile([KC, MT], F32, tag="st")
                        pt = p_pool.tile([KC, MT], F16, tag="pt")
                        nc.tensor.matmul(
                            st[:, coff:],
                            qkt[:, M + kc * KC:M + (kc + 1) * KC],
                            qkt[:, m0 + coff:m0 + MT],
                            start=True,
                            stop=True,
                        )
                        if causal and kc >= 4 * j:
                            # mask this band chunk's leading triangle
                            nc.vector.tensor_add(
                                st[:, coff:coff + KC], st[:, coff:coff + KC],
                                mask_sb,
                            )
                        nc.scalar.activation(
                            pt[:, coff:],
                            st[:, coff:],
                            mybir.ActivationFunctionType.Exp,
                            scale=SCALE,
                        )
                        pts.append((pt, kc, coff))

                    # deferred normalize of the previous m-tile now that this
                    # m-tile's S matmuls are in the PE queue
                    normalize_flush()

                    # phase 2: PV accumulation, split-fp16 (exact products):
                    # o += P16 V_hi + P16 V_lo; ones column of V_hi gives l
                    ot = o_psum.tile([D + 1, MT], F32)
                    for idx, (pt, kc, coff) in enumerate(pts):
                        for vi, vv in enumerate((vah, val)):
                            nc.tensor.matmul(
                                ot[:, coff:],
                                vv[:, p, kc, :],
                                pt[:, coff:],
                                start=(idx == 0 and vi == 0),
                                stop=(idx == len(pts) - 1 and vi == 1),
                            )

                    rl = rl_pool.tile([1, MT], F32, tag="rl")
                    nc.vector.reciprocal(rl, ot[D:D + 1, :])
                    pending.append((p, j, ot, rl))

                if debug_stage or p in done_after:
                    normalize_flush()

                if debug_stage:
                    for s in range(4):
                        sl = slice(s * 16, (s + 1) * 16)
                        nc.gpsimd.dma_start(
                            out=out_d[p, sl, :], in_=stage[sl, p, :]
                        )
                    continue
                # per-head seq-sum renorm + output, as soon as the head is done
                if p in done_after:
                    probs = done_after[p]
                    denom = rl_pool.tile([D, 1], F32, tag="dn")
                    lo, hi = probs[0] * NMT, (probs[-1] + 1) * NMT
                    nc.vector.reduce_sum(
                        out=denom, in_=dsums[:, lo:hi], axis=mybir.AxisListType.X
                    )
                    rden = rl_pool.tile([D, 1], F32, tag="rd")
                    nc.vector.reciprocal(rden, denom)
                    for pp in probs:
                        nc.vector.tensor_scalar_mul(
                            stage[:, pp, :], stage[:, pp, :], rden
                        )
                        # output DMAs on the gpsimd queue (SWDGE casts
                        # fp32->fp16 in flight, halving output bytes) so they
                        # never serialize behind the HWDGE input streams
                        for s in range(4):
                            sl = slice(s * 16, (s + 1) * 16)
                            nc.gpsimd.dma_start(
                                out=out_d[pp, sl, :], in_=stage[sl, pp, :]
                            )
              if debug_stage:
                nc.sync.dma_start(out=dbg_d[:], in_=dsums)

    nc.finalize()
    return nc


def _shard_inputs(query, key, value):
    """Build the 8 per-core input maps from full inputs."""
    in_maps = []
    for core in range(8):
        b = core // 4
        qkT = np.empty((NP, D, 2 * M), np.float32)
        vA = np.empty((KC, NP, NKC, D + 1), np.float32)
        vA[..., D] = 1.0
        for p, (h, pos) in enumerate(_core_problems(core)):
            s, e, st = pos[0], pos[-1] + 1, (pos[1] - pos[0])
            qkT[p, :, :M] = query[b, s:e:st, h, :].T
            qkT[p, :, M:] = key[b, s:e:st, h, :].T
            # vA[i, p, c, :64] = V[c*128 + i]
            vA[:, p, :, :D] = value[b, s:e:st, h, :].reshape(NKC, KC, D).transpose(1, 0, 2)
        vAh = vA.astype(np.float16)
        vAl = (vA - vAh.astype(np.float32)).astype(np.float16)
        in_maps.append({"qkT": qkT, "vAh": vAh, "vAl": vAl})
    return in_maps


def _unshard(results):
    out = np.zeros((B, SEQ, H, D), np.float32)
    for core in range(8):
        b = core // 4
        o = results[core]["out"].astype(np.float32)  # [NP, 64, 2048]
        for p, (h, pos) in enumerate(_core_problems(core)):
            s, e, st = pos[0], pos[-1] + 1, (pos[1] - pos[0])
            out[b, s:e:st, h, :] = o[p].T
    return out


def kernel(query, key, value, causal):
    _import_concourse()
    from concourse.bass_utils import run_bass_kernel_spmd

    query = np.asarray(query, np.float32)
    key = np.asarray(key, np.float32)
    value = np.asarray(value, np.float32)
    causal = bool(int(np.asarray(causal)))

    if causal not in _CACHE:
        _CACHE[causal] = _build_program(causal)
    nc = _CACHE[causal]

    in_maps = _shard_inputs(query, key, value)
    res = run_bass_kernel_spmd(nc, in_maps, core_ids=list(range(8)))
    return _unshard(res.results)
